# revision 35
# baseline (speedup 1.0000x reference)
"""Multi-head attention (B=2, N=2048, C=1024, H=16, D=64) on 8 trn2 NeuronCores.

Returns (out [2,2048,1024] f32, attn [2,16,2048,2048] f32) — matching the
reference nn.Module which returns the attention probabilities as a second
output (512 MB of required HBM writes → near the HBM roofline).

Sharding: head-parallel. Core c computes global heads {2c, 2c+1} for both
batches. Host pre-transposes x and the per-core weight slices (fp16), device
does qkv-proj + flash-style attention + partial output projection; host sums
the 8 partial projections (the "all-reduce") and adds b_proj.

Device-side dataflow per core (all matmul operands fp16, fp32 PSUM accum):
  1. proj: qT/kT/vT [128(2h·64d), 2048] = W.T @ xT, accumulated over 8
     c-tiles. vT is PE-transposed into v_aug [j, d] layout with a ones
     column appended (col 64/130) so the PV matmul also produces the
     softmax denominators.
  2. per (b,h) pass A over 16 j-tiles: scores_T[j,i] = kT_h.T @ qT_h
     (PSUM) → exp via ScalarE (scale=1/32 folded in) → SBUF fp16 exp_all
     (all 16 tiles stay resident) → PV matmul accumulates
     out_aug[65, 2048] = [v|1].T @ exp over j-tiles.
  3. denominators: sums (row 64 of the PV accumulator) are bounced through
     HBM to reshape [1, 2048] → per-partition [128, 16], then one 128-lane
     DVE reciprocal.
  4. pass B over 16 i-chunks: PE-transposes exp blocks back to [i, j]
     orientation (fp16 PSUM); the mandatory PSUM→SBUF evacuate doubles as
     the ×recip normalize (per-partition scalar on DVE/ACT) producing f32
     attn rows; DMA writes 1 MB contiguous blocks.
  5. out-proj per head (K=64 pairs auto-pack into disjoint PE row groups);
     the ×recip(head) normalize rides the PSUM evacuate
     (tensor_scalar + scalar_tensor_tensor merge) → fp16 partial → HBM.

Perf notes: exp buffers ping-pong across (b,h) so pass B(bh) overlaps pass
A(bh+1); the projection phase accumulates through the shared tp PSUM pool so
proj(b1) overlaps batch-0 attention; consecutive j-tile QK matmuls (K=64)
alternate PE row groups via swapped-half copies (qT2/kT2) to pack the
systolic array. PSUM budget (8 banks): qk 2x2 + pv 2 + tp 2x1.
~430 us/core measured (HBM-write floor for the 512 MB attn output is ~180).
"""

import os

import numpy as np

import concourse.mybir as mybir
import concourse.tile as tile
from concourse import bacc
from concourse.bass import ds, ts
from concourse.bass_utils import run_bass_kernel_spmd
from concourse.masks import make_identity

B, N, C, H = 2, 2048, 1024, 16
D = C // H  # 64
SCALE = 1.0 / (C**0.5)
HEADS_PER_CORE = 2  # per batch
N_CORES = 8
NJT = N // 128  # 16 j-tiles
NIC = N // 128  # 16 i-chunks
NCT = C // 128  # 8 contraction tiles for projections
FP16 = mybir.dt.float16
FP32 = mybir.dt.float32

# v_aug free layout per j-tile: [h0 d0..63, h0 ones, pad, h1 d0..63, h1 ones, pad]
VAUG_W = 132
VAUG_H_OFF = (0, 66)


def build_core_program():
    nc = bacc.Bacc(None, target_bir_lowering=False)

    xT = nc.dram_tensor("xT", [B, C, N], FP16, kind="ExternalInput")
    # [C, 384]: cols 0:128 = qT (2 heads), 128:256 = kT, 256:384 = vT
    wqkv = nc.dram_tensor("wqkv", [C, 384], FP16, kind="ExternalInput")
    wproj = nc.dram_tensor("wproj", [128, C], FP16, kind="ExternalInput")

    attn_out = nc.dram_tensor(
        "attn_part", [B, HEADS_PER_CORE, N, N], FP32, kind="ExternalOutput"
    )
    out_part = nc.dram_tensor("out_part", [B, N, C], FP16, kind="ExternalOutput")

    with tile.TileContext(nc) as tc:
        with (
            tc.tile_pool(name="persist", bufs=1) as persist,
            tc.tile_pool(name="stage", bufs=3) as stage,
            tc.tile_pool(name="dram", bufs=2, space="DRAM") as dram,
            tc.tile_pool(name="qkps", bufs=2, space="PSUM") as qkps,
            tc.tile_pool(name="pvps", bufs=1, space="PSUM") as pvps,
            tc.tile_pool(name="tpps", bufs=3, space="PSUM") as tpps,
        ):
            # ---- constants / weights ----
            identity = persist.tile([128, 128], FP16)
            make_identity(nc, identity[:, :])

            wproj_sb = persist.tile([128, C], FP16)
            nc.sync.dma_start(wproj_sb[:, :], wproj[:, :])

            # ---- persistent activations ----
            qT = [persist.tile([128, N], FP16, name=f"qT{b}") for b in range(B)]
            kT = [persist.tile([128, N], FP16, name=f"kT{b}") for b in range(B)]
            v_aug = [
                persist.tile([128, NJT, VAUG_W], FP16, name=f"vaug{b}")
                for b in range(B)
            ]
            exp_all = persist.tile([128, NJT, N], FP16)
            stacked_t = persist.tile([128, N], FP16)
            stacked = [stacked_t, stacked_t]
            qT2 = [persist.tile([128, N], FP16, name=f"qT2{b}") for b in range(B)]
            kT2 = [persist.tile([128, N], FP16, name=f"kT2{b}") for b in range(B)]
            recip_cols = [
                [
                    persist.tile([128, NIC], FP32, name=f"rcols{b}_{hh}")
                    for hh in range(HEADS_PER_CORE)
                ]
                for b in range(B)
            ]

            for b in range(B):
                for hh in range(HEADS_PER_CORE):
                    nc.vector.memset(v_aug[b][:, :, VAUG_H_OFF[hh] + 64], 1.0)

            # ================= projection phase =================
            # Accumulates through the shared tp PSUM pool (quarter tiles) so
            # proj(b1) overlaps batch-0 attention instead of serializing on
            # a dedicated pool.
            with tc.tile_pool(name="xtp", bufs=8) as xtp:
                w_sb = xtp.tile([128, NCT, 384], FP16, tag="w_sb", bufs=1)
                nc.sync.dma_start(
                    w_sb[:, :, :], wqkv.rearrange("(t p) w -> p t w", p=128)
                )
                for b in range(B):
                    xts = []
                    for ct in range(NCT):
                        xt = xtp.tile([128, N], FP16, tag="xt")
                        nc.sync.dma_start(xt[:, :], xT[b, ts(ct, 128), :])
                        xts.append(xt)

                    # qT, kT, vT accumulated in [128, 512] quarters
                    vT_sb = xtp.tile([128, N], FP16, tag="vT_sb", bufs=2)
                    for ti, dest in ((0, qT[b]), (1, kT[b]), (2, vT_sb)):
                        for qt in range(4):
                            ps = tpps.tile(
                                [128, 512], FP32, tag="tp", name=f"pj{b}_{ti}_{qt}"
                            )
                            for ct in range(NCT):
                                nc.tensor.matmul(
                                    ps[:, :],
                                    w_sb[:, ct, ts(ti, 128)],
                                    xts[ct][:, ts(qt, 512)],
                                    start=(ct == 0),
                                    stop=(ct == NCT - 1),
                                )
                            nc.scalar.copy(dest[:, ts(qt, 512)], ps[:, :])

                    # v_aug: PE-transpose vT 128x128 blocks into [j, d] layout
                    for jg in range(2):
                        tp = tpps.tile([128, 1024], FP16, tag="tp", name=f"vt{b}_{jg}")
                        for k in range(8):
                            jt = jg * 8 + k
                            nc.tensor.transpose(
                                tp[:, ts(k, 128)], vT_sb[:, ts(jt, 128)], identity[:, :]
                            )
                        for k in range(8):
                            jt = jg * 8 + k
                            for hh in range(HEADS_PER_CORE):
                                nc.vector.tensor_copy(
                                    v_aug[b][:, jt, ds(VAUG_H_OFF[hh], 64)],
                                    tp[:, ds(k * 128 + hh * 64, 64)],
                                )
                    # swapped-half copies so consecutive j-tiles use disjoint
                    # PE row groups (K=64 matmuls run concurrently)
                    for src_t, dst_t in ((qT[b], qT2[b]), (kT[b], kT2[b])):
                        nc.vector.tensor_copy(dst_t[0:64, :], src_t[64:128, :])
                        nc.vector.tensor_copy(dst_t[64:128, :], src_t[0:64, :])

            # ================= attention =================
            # PSUM: qk [128,1024]f32 x2 = 4 banks, pv [65,1024]f32 x1 = 2,
            # tp ([128,1024]fp16 | [128,512]f32) x2 = 2. QK j-tile pairs use
            # alternating PE row groups (via qT2/kT2 swapped copies) so the
            # K=64 matmuls pack 2x into the array.
            with tc.tile_pool(name="exp2p", bufs=1) as exp2p:
                exp_b = exp2p.tile([128, NJT, N], FP16)
                evac_cnt = 0
                for bh in range(B * HEADS_PER_CORE):
                    b, hh = divmod(bh, HEADS_PER_CORE)
                    ecur = exp_all if bh % 2 == 0 else exp_b

                    def qk_ops(jt):
                        par = jt % 2
                        row = ds(hh * 64 if par == 0 else (1 - hh) * 64, 64)
                        qsrc = qT[b] if par == 0 else qT2[b]
                        ksrc = kT[b] if par == 0 else kT2[b]
                        return ksrc[row, ts(jt, 128)], qsrc, row

                    # ---- pass A ----
                    pv0 = pvps.tile([65, 512], FP32, tag="pv", name=f"pv0_{bh}")
                    for jp in range(NJT // 2):
                        jts = (2 * jp, 2 * jp + 1)
                        for ih in range(2):
                            qks = {}
                            for jt in jts:
                                qks[jt] = qkps.tile(
                                    [128, 1024], FP32, tag="qk", name=f"qk{bh}_{jt}_{ih}"
                                )
                            for q2 in range(2):
                                for jt in jts:
                                    klhs, qsrc, row = qk_ops(jt)
                                    nc.tensor.matmul(
                                        qks[jt][:, ts(q2, 512)],
                                        klhs,
                                        qsrc[row, ds(ih * 1024 + q2 * 512, 512)],
                                        start=True,
                                        stop=True,
                                    )
                            for jt in jts:
                                nc.scalar.activation(
                                    ecur[:, jt, ts(ih, 1024)],
                                    qks[jt][:, :],
                                    mybir.ActivationFunctionType.Exp,
                                    scale=float(SCALE),
                                )
                        for jt in jts:
                            nc.tensor.matmul(
                                pv0[:, :],
                                v_aug[b][:, jt, ds(VAUG_H_OFF[hh], 65)],
                                ecur[:, jt, 0:512],
                                start=(jt == 0),
                                stop=(jt == NJT - 1),
                            )

                    # ---- PV i-quarters 1..3 + denominators ----
                    sums_row = stage.tile([1, N], FP16, tag="sums_row", bufs=1)
                    nc.scalar.copy(sums_row[:, 0:512], pv0[64:65, :])
                    nc.vector.tensor_copy(stacked[b][ds(hh * 64, 64), 0:512], pv0[0:64, :])
                    for qt in range(1, 4):
                        pvq = pvps.tile(
                            [65, 512], FP32, tag="pv", name=f"pv{qt}_{bh}"
                        )
                        for jt in range(NJT):
                            nc.tensor.matmul(
                                pvq[:, :],
                                v_aug[b][:, jt, ds(VAUG_H_OFF[hh], 65)],
                                ecur[:, jt, ts(qt, 512)],
                                start=(jt == 0),
                                stop=(jt == NJT - 1),
                            )
                        nc.scalar.copy(sums_row[:, ts(qt, 512)], pvq[64:65, :])
                        nc.vector.tensor_copy(
                            stacked[b][ds(hh * 64, 64), ts(qt, 512)], pvq[0:64, :]
                        )
                    rcols = recip_cols[b][hh]
                    bounce = dram.tile([1, N], FP16, tag="bounce")
                    nc.sync.dma_start(bounce[:, :], sums_row[:, :])
                    sums_cols = stage.tile([128, NIC], FP16, tag="sums_cols", bufs=2)
                    nc.sync.dma_start(
                        sums_cols[:, :],
                        bounce.rearrange("o (c p) -> (o p) c", p=128),
                    )
                    nc.vector.reciprocal(rcols[:, :], sums_cols[:, :])

                    # ---- output projection (tp pool, packed K=64 pairs) ----
                    if hh == 1:
                        for ic in range(NIC):
                            out_sb = stage.tile([128, C], FP16, tag="out_sb", bufs=2)
                            for nh in range(2):
                                ps = [
                                    tpps.tile(
                                        [128, 512],
                                        FP32,
                                        tag="tp",
                                        name=f"op{b}_{ic}_{nh}_{h2}",
                                    )
                                    for h2 in range(HEADS_PER_CORE)
                                ]
                                for h2 in range(HEADS_PER_CORE):
                                    nc.tensor.matmul(
                                        ps[h2][:, :],
                                        stacked[b][ds(h2 * 64, 64), ts(ic, 128)],
                                        wproj_sb[ds(h2 * 64, 64), ts(nh, 512)],
                                        start=True,
                                        stop=True,
                                    )
                                optmp = stage.tile(
                                    [128, 512], FP16, tag="optmp", bufs=1
                                )
                                if b == 1:
                                    nc.scalar.mul(
                                        optmp[:, :],
                                        ps[0][:, :],
                                        recip_cols[b][0][:, ic : ic + 1],
                                    )
                                else:
                                    nc.vector.tensor_scalar_mul(
                                        optmp[:, :],
                                        ps[0][:, :],
                                        recip_cols[b][0][:, ic : ic + 1],
                                    )
                                nc.vector.scalar_tensor_tensor(
                                    out_sb[:, ts(nh, 512)],
                                    ps[1][:, :],
                                    recip_cols[b][1][:, ic : ic + 1],
                                    optmp[:, :],
                                    mybir.AluOpType.mult,
                                    mybir.AluOpType.add,
                                )
                            nc.sync.dma_start(
                                out_part[b, ts(ic, 128), :], out_sb[:, :]
                            )

                    # ---- pass B ----
                    for ic in range(NIC):
                        attn_sb = stage.tile([128, N], FP32, tag="attn_sb", bufs=3)
                        for jh in range(2):
                            tp = tpps.tile(
                                [128, 1024], FP16, tag="tp", name=f"tp{bh}_{ic}_{jh}"
                            )
                            for k in range(8):
                                jt = jh * 8 + k
                                nc.tensor.transpose(
                                    tp[:, ts(k, 128)],
                                    ecur[:, jt, ts(ic, 128)],
                                    identity[:, :],
                                )
                            act_evac = (
                                evac_cnt % 2 == 1 if bh == 3 else evac_cnt % 4 == 3
                            )
                            if act_evac:
                                nc.scalar.mul(
                                    attn_sb[:, ts(jh, 1024)],
                                    tp[:, :],
                                    rcols[:, ic : ic + 1],
                                )
                            else:
                                nc.vector.tensor_scalar_mul(
                                    attn_sb[:, ts(jh, 1024)],
                                    tp[:, :],
                                    rcols[:, ic : ic + 1],
                                )
                            evac_cnt += 1
                        nc.sync.dma_start(
                            attn_out[b, hh, ts(ic, 128), :], attn_sb[:, :]
                        )

    nc.finalize()
    return nc


def _prep_inputs(x, w_qkv, w_proj):
    xT = np.ascontiguousarray(x.transpose(0, 2, 1)).astype(np.float16)
    wq, wk, wv = w_qkv[0:C], w_qkv[C : 2 * C], w_qkv[2 * C : 3 * C]
    in_maps = []
    for c in range(N_CORES):
        sl = slice(c * 128, (c + 1) * 128)
        wqkv_c = np.concatenate(
            [wq[sl].T, wk[sl].T, wv[sl].T], axis=1
        ).astype(np.float16)
        wproj_c = np.ascontiguousarray(w_proj[:, sl].T).astype(np.float16)
        in_maps.append(
            {
                "xT": xT,
                "wqkv": np.ascontiguousarray(wqkv_c),
                "wproj": wproj_c,
            }
        )
    return in_maps


def _install_ntff_shim():
    """The agent image lacks antenv.axon_hooks; recreate the NTFF profile
    hook via ctypes against libaxon_pjrt.so (same ABI trn_boot uses)."""
    import contextlib
    import ctypes
    import sys
    import types

    try:
        from antenv.axon_hooks import get_axon_ntff_profile_hook  # noqa: F401

        return
    except ImportError:
        pass
    so_path = "/opt/axon/libaxon_pjrt.so"
    if not os.path.exists(so_path):
        return
    lib = ctypes.CDLL(so_path)
    if not hasattr(lib, "axon_start_nrt_profile"):
        return
    lib.axon_start_nrt_profile.argtypes = [
        ctypes.POINTER(ctypes.c_int64),
        ctypes.c_size_t,
    ]
    lib.axon_start_nrt_profile.restype = ctypes.c_int64
    lib.axon_stop_nrt_profile.argtypes = [ctypes.c_char_p]
    lib.axon_stop_nrt_profile.restype = ctypes.c_int64

    @contextlib.contextmanager
    def _hook(output_dir, device_ids):
        import jax

        jax.devices()
        if device_ids:
            ids = (ctypes.c_int64 * len(device_ids))(*device_ids)
            rc = lib.axon_start_nrt_profile(ids, len(device_ids))
        else:
            rc = lib.axon_start_nrt_profile(None, 0)
        if rc != 0:
            raise RuntimeError(f"axon_start_nrt_profile rc={rc}")
        try:
            yield
        finally:
            n = lib.axon_stop_nrt_profile(str(output_dir).encode())
            print(f"profile: {n} file(s) written to {output_dir}")

    mod = types.ModuleType("antenv.axon_hooks")
    mod.get_axon_ntff_profile_hook = lambda: _hook
    mod.set_axon_ntff_profile_hook = lambda h: None
    sys.modules["antenv.axon_hooks"] = mod


_CACHED = {}


def kernel(x, w_qkv, w_proj, b_proj):
    x = np.asarray(x)
    w_qkv = np.asarray(w_qkv)
    w_proj = np.asarray(w_proj)
    b_proj = np.asarray(b_proj)

    if "nc" not in _CACHED:
        _CACHED["nc"] = build_core_program()
    nc = _CACHED["nc"]

    in_maps = _prep_inputs(x, w_qkv, w_proj)
    trace = os.environ.get("KERNEL_TRACE", "") not in ("", "0")
    if trace:
        _install_ntff_shim()
        import concourse.bass_utils as _bu

        _bu.upload_artifacts = lambda d: d  # zero-egress container
    res = run_bass_kernel_spmd(
        nc, in_maps, core_ids=list(range(N_CORES)), trace=trace
    )
    if trace and res.exec_time_ns is not None:
        print(f"HW exec time: {res.exec_time_ns} ns")
        if res.instructions_and_trace:
            print(f"trace: {res.instructions_and_trace[1]}")

    attn = np.empty((B, H, N, N), dtype=np.float32)
    out = np.zeros((B, N, C), dtype=np.float32)
    for c in range(N_CORES):
        r = res.results[c]
        ap = r["attn_part"]
        for hh in range(HEADS_PER_CORE):
            attn[:, HEADS_PER_CORE * c + hh] = ap[:, hh]
        out += r["out_part"].astype(np.float32)
    out += b_proj.astype(np.float32)
    return out, attn


# revision 36
# speedup vs baseline: 1.0713x; 1.0713x over previous
"""Multi-head attention (B=2, N=2048, C=1024, H=16, D=64) on 8 trn2 NeuronCores.

Returns (out [2,2048,1024] f32, attn [2,16,2048,2048] f32) — matching the
reference nn.Module which returns the attention probabilities as a second
output (512 MB of required HBM writes → near the HBM roofline).

Sharding: head-parallel. Core c computes global heads {2c, 2c+1} for both
batches. Host pre-transposes x and the per-core weight slices (fp16), device
does qkv-proj + flash-style attention + partial output projection; host sums
the 8 partial projections (the "all-reduce") and adds b_proj.

Device-side dataflow per core (all matmul operands fp16, fp32 PSUM accum):
  1. proj: qT/kT/vT [128(2h·64d), 2048] = W.T @ xT, accumulated over 8
     c-tiles. vT is PE-transposed into v_aug [j, d] layout with a ones
     column appended (col 64/130) so the PV matmul also produces the
     softmax denominators.
  2. per (b,h) pass A over 16 j-tiles: scores_T[j,i] = kT_h.T @ qT_h
     (PSUM) → exp via ScalarE (scale=1/32 folded in) → SBUF fp16 exp_all
     (all 16 tiles stay resident) → PV matmul accumulates
     out_aug[65, 2048] = [v|1].T @ exp over j-tiles.
  3. denominators: sums (row 64 of the PV accumulator) are bounced through
     HBM to reshape [1, 2048] → per-partition [128, 16], then one 128-lane
     DVE reciprocal.
  4. pass B over 16 i-chunks: PE-transposes exp blocks back to [i, j]
     orientation (fp16 PSUM); the mandatory PSUM→SBUF evacuate doubles as
     the ×recip normalize (per-partition scalar on DVE/ACT) producing f32
     attn rows; DMA writes 1 MB contiguous blocks.
  5. out-proj per head (K=64 pairs auto-pack into disjoint PE row groups);
     the ×recip(head) normalize rides the PSUM evacuate
     (tensor_scalar + scalar_tensor_tensor merge) → fp16 partial → HBM.

Perf notes: exp buffers ping-pong across (b,h) so pass B(bh) overlaps pass
A(bh+1); the projection phase accumulates through the shared tp PSUM pool so
proj(b1) overlaps batch-0 attention; consecutive j-tile QK matmuls (K=64)
alternate PE row groups via swapped-half copies (qT2/kT2) to pack the
systolic array. PSUM budget (8 banks): qk 2x2 + pv 2 + tp 2x1.
~430 us/core measured (HBM-write floor for the 512 MB attn output is ~180).
"""

import os

import numpy as np

import concourse.mybir as mybir
import concourse.tile as tile
from concourse import bacc
from concourse.bass import ds, ts
from concourse.bass_utils import run_bass_kernel_spmd
from concourse.masks import make_identity

B, N, C, H = 2, 2048, 1024, 16
D = C // H  # 64
SCALE = 1.0 / (C**0.5)
HEADS_PER_CORE = 2  # per batch
N_CORES = 8
NJT = N // 128  # 16 j-tiles
NIC = N // 128  # 16 i-chunks
NCT = C // 128  # 8 contraction tiles for projections
FP16 = mybir.dt.float16
FP32 = mybir.dt.float32

# v_aug free layout per j-tile: [h0 d0..63, h0 ones, pad, h1 d0..63, h1 ones, pad]
VAUG_W = 132
VAUG_H_OFF = (0, 66)


def build_core_program():
    nc = bacc.Bacc(None, target_bir_lowering=False)

    xT = nc.dram_tensor("xT", [B, C, N], FP16, kind="ExternalInput")
    # [C, 384]: cols 0:128 = qT (2 heads), 128:256 = kT, 256:384 = vT
    wqkv = nc.dram_tensor("wqkv", [C, 384], FP16, kind="ExternalInput")
    wproj = nc.dram_tensor("wproj", [128, C], FP16, kind="ExternalInput")

    attn_out = nc.dram_tensor(
        "attn_part", [B, HEADS_PER_CORE, N, N], FP32, kind="ExternalOutput"
    )
    out_part = nc.dram_tensor("out_part", [B, N, C], FP16, kind="ExternalOutput")

    with tile.TileContext(nc) as tc:
        with (
            tc.tile_pool(name="persist", bufs=1) as persist,
            tc.tile_pool(name="stage", bufs=3) as stage,
            tc.tile_pool(name="dram", bufs=2, space="DRAM") as dram,
            tc.tile_pool(name="qkps", bufs=2, space="PSUM") as qkps,
            tc.tile_pool(name="pvps", bufs=1, space="PSUM") as pvps,
            tc.tile_pool(name="tpps", bufs=2, space="PSUM") as tpps,
        ):
            # ---- constants / weights ----
            identity = persist.tile([128, 128], FP16)
            make_identity(nc, identity[:, :])

            wproj_sb = persist.tile([128, C], FP16)
            nc.sync.dma_start(wproj_sb[:, :], wproj[:, :])

            # ---- persistent activations ----
            qT = [persist.tile([128, N], FP16, name=f"qT{b}") for b in range(B)]
            kT = [persist.tile([128, N], FP16, name=f"kT{b}") for b in range(B)]
            v_aug = [
                persist.tile([128, NJT, VAUG_W], FP16, name=f"vaug{b}")
                for b in range(B)
            ]
            exp_all = persist.tile([128, NJT, N], FP16)
            stacked_t = persist.tile([128, N], FP16)
            stacked = [stacked_t, stacked_t]
            qT2 = [persist.tile([128, N], FP16, name=f"qT2{b}") for b in range(B)]
            kT2 = [persist.tile([128, N], FP16, name=f"kT2{b}") for b in range(B)]
            recip_cols = [
                [
                    persist.tile([128, NIC], FP32, name=f"rcols{b}_{hh}")
                    for hh in range(HEADS_PER_CORE)
                ]
                for b in range(B)
            ]

            for b in range(B):
                for hh in range(HEADS_PER_CORE):
                    nc.vector.memset(v_aug[b][:, :, VAUG_H_OFF[hh] + 64], 1.0)

            # ================= projection phase =================
            # Accumulates through the shared tp PSUM pool (quarter tiles) so
            # proj(b1) overlaps batch-0 attention instead of serializing on
            # a dedicated pool.
            with tc.tile_pool(name="xtp", bufs=8) as xtp:
                w_sb = xtp.tile([128, NCT, 384], FP16, tag="w_sb", bufs=1)
                nc.sync.dma_start(
                    w_sb[:, :, :], wqkv.rearrange("(t p) w -> p t w", p=128)
                )
                for b in range(B):
                    xts = []
                    for ct in range(NCT):
                        xt = xtp.tile([128, N], FP16, tag="xt")
                        nc.sync.dma_start(xt[:, :], xT[b, ts(ct, 128), :])
                        xts.append(xt)

                    # qT, kT, vT accumulated in [128, 512] quarters
                    vT_sb = xtp.tile([128, N], FP16, tag="vT_sb", bufs=2)
                    for ti, dest in ((0, qT[b]), (1, kT[b]), (2, vT_sb)):
                        for qt in range(4):
                            ps = tpps.tile(
                                [128, 512], FP32, tag="tp", name=f"pj{b}_{ti}_{qt}"
                            )
                            for ct in range(NCT):
                                nc.tensor.matmul(
                                    ps[:, :],
                                    w_sb[:, ct, ts(ti, 128)],
                                    xts[ct][:, ts(qt, 512)],
                                    start=(ct == 0),
                                    stop=(ct == NCT - 1),
                                )
                            nc.scalar.copy(dest[:, ts(qt, 512)], ps[:, :])

                    # v_aug: PE-transpose vT 128x128 blocks into [j, d] layout
                    for jg in range(2):
                        tp = tpps.tile([128, 1024], FP16, tag="tp", name=f"vt{b}_{jg}")
                        for k in range(8):
                            jt = jg * 8 + k
                            nc.tensor.transpose(
                                tp[:, ts(k, 128)], vT_sb[:, ts(jt, 128)], identity[:, :]
                            )
                        for k in range(8):
                            jt = jg * 8 + k
                            for hh in range(HEADS_PER_CORE):
                                nc.vector.tensor_copy(
                                    v_aug[b][:, jt, ds(VAUG_H_OFF[hh], 64)],
                                    tp[:, ds(k * 128 + hh * 64, 64)],
                                )
                    # swapped-half copies so consecutive j-tiles use disjoint
                    # PE row groups (K=64 matmuls run concurrently)
                    for src_t, dst_t in ((qT[b], qT2[b]), (kT[b], kT2[b])):
                        nc.vector.tensor_copy(dst_t[0:64, :], src_t[64:128, :])
                        nc.vector.tensor_copy(dst_t[64:128, :], src_t[0:64, :])

            # ================= attention =================
            # PSUM: qk [128,1024]f32 x2 = 4 banks, pv [65,1024]f32 x1 = 2,
            # tp ([128,1024]fp16 | [128,512]f32) x2 = 2. QK j-tile pairs use
            # alternating PE row groups (via qT2/kT2 swapped copies) so the
            # K=64 matmuls pack 2x into the array.
            with tc.tile_pool(name="exp2p", bufs=1) as exp2p:
                exp_b = exp2p.tile([128, NJT, N], FP16)
                evac_cnt = 0
                for bh in range(B * HEADS_PER_CORE):
                    b, hh = divmod(bh, HEADS_PER_CORE)
                    ecur = exp_all if bh % 2 == 0 else exp_b

                    def qk_ops(jt):
                        par = jt % 2
                        row = ds(hh * 64 if par == 0 else (1 - hh) * 64, 64)
                        qsrc = qT[b] if par == 0 else qT2[b]
                        ksrc = kT[b] if par == 0 else kT2[b]
                        return ksrc[row, ts(jt, 128)], qsrc, row

                    # ---- pass A ----
                    pv0 = pvps.tile([65, 1024], FP32, tag="pv", name=f"pv0_{bh}")
                    for jp in range(NJT // 2):
                        jts = (2 * jp, 2 * jp + 1)
                        for ih in range(2):
                            qks = {}
                            for jt in jts:
                                qks[jt] = qkps.tile(
                                    [128, 1024], FP32, tag="qk", name=f"qk{bh}_{jt}_{ih}"
                                )
                            for q2 in range(2):
                                for jt in jts:
                                    klhs, qsrc, row = qk_ops(jt)
                                    nc.tensor.matmul(
                                        qks[jt][:, ts(q2, 512)],
                                        klhs,
                                        qsrc[row, ds(ih * 1024 + q2 * 512, 512)],
                                        start=True,
                                        stop=True,
                                    )
                            for jt in jts:
                                nc.scalar.activation(
                                    ecur[:, jt, ts(ih, 1024)],
                                    qks[jt][:, :],
                                    mybir.ActivationFunctionType.Exp,
                                    scale=float(SCALE),
                                )
                        for jt in jts:
                            for q2 in range(2):
                                nc.tensor.matmul(
                                    pv0[:, ts(q2, 512)],
                                    v_aug[b][:, jt, ds(VAUG_H_OFF[hh], 65)],
                                    ecur[:, jt, ts(q2, 512)],
                                    start=(jt == 0),
                                    stop=(jt == NJT - 1),
                                )

                    # ---- PV i-half 1 + denominators ----
                    sums_row = stage.tile([1, N], FP32, tag="sums_row", bufs=1)
                    nc.scalar.copy(sums_row[:, 0:1024], pv0[64:65, :])
                    nc.vector.tensor_copy(stacked[b][ds(hh * 64, 64), 0:1024], pv0[0:64, :])
                    pv1 = pvps.tile([65, 1024], FP32, tag="pv", name=f"pv1_{bh}")
                    for jt in range(NJT):
                        for q2 in range(2):
                            nc.tensor.matmul(
                                pv1[:, ts(q2, 512)],
                                v_aug[b][:, jt, ds(VAUG_H_OFF[hh], 65)],
                                ecur[:, jt, ds(1024 + q2 * 512, 512)],
                                start=(jt == 0),
                                stop=(jt == NJT - 1),
                            )
                    nc.scalar.copy(sums_row[:, 1024:2048], pv1[64:65, :])
                    nc.vector.tensor_copy(
                        stacked[b][ds(hh * 64, 64), 1024:2048], pv1[0:64, :]
                    )
                    rcols = recip_cols[b][hh]
                    bounce = dram.tile([1, N], FP32, tag="bounce")
                    nc.sync.dma_start(bounce[:, :], sums_row[:, :])
                    sums_cols = stage.tile([128, NIC], FP32, tag="sums_cols", bufs=2)
                    nc.sync.dma_start(
                        sums_cols[:, :],
                        bounce.rearrange("o (c p) -> (o p) c", p=128),
                    )
                    nc.vector.reciprocal(rcols[:, :], sums_cols[:, :])

                    # ---- output projection (tp pool, packed K=64 pairs) ----
                    if hh == 1:
                        for ic in range(NIC):
                            ps = []
                            for h2 in range(HEADS_PER_CORE):
                                p = pvps.tile(
                                    [128, 1024], FP32, tag="pv", name=f"op{b}_{ic}_{h2}"
                                )
                                for q2 in range(2):
                                    nc.tensor.matmul(
                                        p[:, ts(q2, 512)],
                                        stacked[b][ds(h2 * 64, 64), ts(ic, 128)],
                                        wproj_sb[ds(h2 * 64, 64), ts(q2, 512)],
                                        start=True,
                                        stop=True,
                                    )
                                ps.append(p)
                            optmp = stage.tile([128, C], FP16, tag="optmp", bufs=2)
                            if b == 1:
                                nc.scalar.mul(
                                    optmp[:, :],
                                    ps[0][:, :],
                                    recip_cols[b][0][:, ic : ic + 1],
                                )
                            else:
                                nc.vector.tensor_scalar_mul(
                                    optmp[:, :],
                                    ps[0][:, :],
                                    recip_cols[b][0][:, ic : ic + 1],
                                )
                            out_sb = stage.tile([128, C], FP16, tag="out_sb", bufs=2)
                            nc.vector.scalar_tensor_tensor(
                                out_sb[:, :],
                                ps[1][:, :],
                                recip_cols[b][1][:, ic : ic + 1],
                                optmp[:, :],
                                mybir.AluOpType.mult,
                                mybir.AluOpType.add,
                            )
                            nc.sync.dma_start(
                                out_part[b, ts(ic, 128), :], out_sb[:, :]
                            )

                    # ---- pass B ----
                    for ic in range(NIC):
                        attn_sb = stage.tile([128, N], FP32, tag="attn_sb", bufs=2)
                        for jh in range(2):
                            tp = tpps.tile(
                                [128, 1024], FP16, tag="tp", name=f"tp{bh}_{ic}_{jh}"
                            )
                            for k in range(8):
                                jt = jh * 8 + k
                                nc.tensor.transpose(
                                    tp[:, ts(k, 128)],
                                    ecur[:, jt, ts(ic, 128)],
                                    identity[:, :],
                                )
                            act_evac = (
                                evac_cnt % 2 == 1 if bh == 3 else evac_cnt % 4 == 3
                            )
                            if act_evac:
                                nc.scalar.mul(
                                    attn_sb[:, ts(jh, 1024)],
                                    tp[:, :],
                                    rcols[:, ic : ic + 1],
                                )
                            else:
                                nc.vector.tensor_scalar_mul(
                                    attn_sb[:, ts(jh, 1024)],
                                    tp[:, :],
                                    rcols[:, ic : ic + 1],
                                )
                            evac_cnt += 1
                        nc.sync.dma_start(
                            attn_out[b, hh, ts(ic, 128), :], attn_sb[:, :]
                        )

    nc.finalize()
    return nc


def _prep_inputs(x, w_qkv, w_proj):
    xT = np.ascontiguousarray(x.transpose(0, 2, 1)).astype(np.float16)
    wq, wk, wv = w_qkv[0:C], w_qkv[C : 2 * C], w_qkv[2 * C : 3 * C]
    in_maps = []
    for c in range(N_CORES):
        sl = slice(c * 128, (c + 1) * 128)
        wqkv_c = np.concatenate(
            [wq[sl].T, wk[sl].T, wv[sl].T], axis=1
        ).astype(np.float16)
        wproj_c = np.ascontiguousarray(w_proj[:, sl].T).astype(np.float16)
        in_maps.append(
            {
                "xT": xT,
                "wqkv": np.ascontiguousarray(wqkv_c),
                "wproj": wproj_c,
            }
        )
    return in_maps


def _install_ntff_shim():
    """The agent image lacks antenv.axon_hooks; recreate the NTFF profile
    hook via ctypes against libaxon_pjrt.so (same ABI trn_boot uses)."""
    import contextlib
    import ctypes
    import sys
    import types

    try:
        from antenv.axon_hooks import get_axon_ntff_profile_hook  # noqa: F401

        return
    except ImportError:
        pass
    so_path = "/opt/axon/libaxon_pjrt.so"
    if not os.path.exists(so_path):
        return
    lib = ctypes.CDLL(so_path)
    if not hasattr(lib, "axon_start_nrt_profile"):
        return
    lib.axon_start_nrt_profile.argtypes = [
        ctypes.POINTER(ctypes.c_int64),
        ctypes.c_size_t,
    ]
    lib.axon_start_nrt_profile.restype = ctypes.c_int64
    lib.axon_stop_nrt_profile.argtypes = [ctypes.c_char_p]
    lib.axon_stop_nrt_profile.restype = ctypes.c_int64

    @contextlib.contextmanager
    def _hook(output_dir, device_ids):
        import jax

        jax.devices()
        if device_ids:
            ids = (ctypes.c_int64 * len(device_ids))(*device_ids)
            rc = lib.axon_start_nrt_profile(ids, len(device_ids))
        else:
            rc = lib.axon_start_nrt_profile(None, 0)
        if rc != 0:
            raise RuntimeError(f"axon_start_nrt_profile rc={rc}")
        try:
            yield
        finally:
            n = lib.axon_stop_nrt_profile(str(output_dir).encode())
            print(f"profile: {n} file(s) written to {output_dir}")

    mod = types.ModuleType("antenv.axon_hooks")
    mod.get_axon_ntff_profile_hook = lambda: _hook
    mod.set_axon_ntff_profile_hook = lambda h: None
    sys.modules["antenv.axon_hooks"] = mod


_CACHED = {}


def kernel(x, w_qkv, w_proj, b_proj):
    x = np.asarray(x)
    w_qkv = np.asarray(w_qkv)
    w_proj = np.asarray(w_proj)
    b_proj = np.asarray(b_proj)

    if "nc" not in _CACHED:
        _CACHED["nc"] = build_core_program()
    nc = _CACHED["nc"]

    in_maps = _prep_inputs(x, w_qkv, w_proj)
    trace = os.environ.get("KERNEL_TRACE", "") not in ("", "0")
    if trace:
        _install_ntff_shim()
        import concourse.bass_utils as _bu

        _bu.upload_artifacts = lambda d: d  # zero-egress container
    res = run_bass_kernel_spmd(
        nc, in_maps, core_ids=list(range(N_CORES)), trace=trace
    )
    if trace and res.exec_time_ns is not None:
        print(f"HW exec time: {res.exec_time_ns} ns")
        if res.instructions_and_trace:
            print(f"trace: {res.instructions_and_trace[1]}")

    attn = np.empty((B, H, N, N), dtype=np.float32)
    out = np.zeros((B, N, C), dtype=np.float32)
    for c in range(N_CORES):
        r = res.results[c]
        ap = r["attn_part"]
        for hh in range(HEADS_PER_CORE):
            attn[:, HEADS_PER_CORE * c + hh] = ap[:, hh]
        out += r["out_part"].astype(np.float32)
    out += b_proj.astype(np.float32)
    return out, attn


# revision 37
# speedup vs baseline: 1.1409x; 1.0650x over previous
"""Multi-head attention (B=2, N=2048, C=1024, H=16, D=64) on 8 trn2 NeuronCores.

Returns (out [2,2048,1024] f32, attn [2,16,2048,2048] f32) — matching the
reference nn.Module which returns the attention probabilities as a second
output (512 MB of required HBM writes → near the HBM roofline).

Sharding: head-parallel. Core c computes global heads {2c, 2c+1} for both
batches. Host pre-transposes x and the per-core weight slices (fp16), device
does qkv-proj + flash-style attention + partial output projection; host sums
the 8 partial projections (the "all-reduce") and adds b_proj.

Device-side dataflow per core (all matmul operands fp16, fp32 PSUM accum):
  1. proj: qT/kT/vT [128(2h·64d), 2048] = W.T @ xT, accumulated over 8
     c-tiles. vT is PE-transposed into v_aug [j, d] layout with a ones
     column appended (col 64/130) so the PV matmul also produces the
     softmax denominators.
  2. per (b,h) pass A over 16 j-tiles: scores_T[j,i] = kT_h.T @ qT_h
     (PSUM) → exp via ScalarE (scale=1/32 folded in) → SBUF fp16 exp_all
     (all 16 tiles stay resident) → PV matmul accumulates
     out_aug[65, 2048] = [v|1].T @ exp over j-tiles.
  3. denominators: sums (row 64 of the PV accumulator) are bounced through
     HBM to reshape [1, 2048] → per-partition [128, 16], then one 128-lane
     DVE reciprocal.
  4. pass B over 16 i-chunks: PE-transposes exp blocks back to [i, j]
     orientation (fp16 PSUM); the mandatory PSUM→SBUF evacuate doubles as
     the ×recip normalize (per-partition scalar on DVE/ACT) producing f32
     attn rows; DMA writes 1 MB contiguous blocks.
  5. out-proj per head (K=64 pairs auto-pack into disjoint PE row groups);
     the ×recip(head) normalize rides the PSUM evacuate
     (tensor_scalar + scalar_tensor_tensor merge) → fp16 partial → HBM.

Perf notes: exp buffers ping-pong across (b,h) so pass B(bh) overlaps pass
A(bh+1); the projection phase accumulates through the shared tp PSUM pool so
proj(b1) overlaps batch-0 attention; consecutive j-tile QK matmuls (K=64)
alternate PE row groups via swapped-half copies (qT2/kT2) to pack the
systolic array. PSUM budget (8 banks): qk 2x2 + pv 2 + tp 2x1.
~430 us/core measured (HBM-write floor for the 512 MB attn output is ~180).
"""

import os

import numpy as np

import concourse.mybir as mybir
import concourse.tile as tile
from concourse import bacc
from concourse.bass import ds, ts
from concourse.bass_utils import run_bass_kernel_spmd
from concourse.masks import make_identity

B, N, C, H = 2, 2048, 1024, 16
D = C // H  # 64
SCALE = 1.0 / (C**0.5)
HEADS_PER_CORE = 2  # per batch
N_CORES = 8
NJT = N // 128  # 16 j-tiles
NIC = N // 128  # 16 i-chunks
NCT = C // 128  # 8 contraction tiles for projections
FP16 = mybir.dt.float16
FP32 = mybir.dt.float32

# v_aug free layout per j-tile: [h0 d0..63, h0 ones, pad, h1 d0..63, h1 ones, pad]
VAUG_W = 132
VAUG_H_OFF = (0, 66)


def build_core_program():
    nc = bacc.Bacc(None, target_bir_lowering=False)

    xT = nc.dram_tensor("xT", [B, C, N], FP16, kind="ExternalInput")
    # [C, 384]: cols 0:128 = qT (2 heads), 128:256 = kT, 256:384 = vT
    wqkv = nc.dram_tensor("wqkv", [C, 384], FP16, kind="ExternalInput")
    wproj = nc.dram_tensor("wproj", [128, C], FP16, kind="ExternalInput")

    attn_out = nc.dram_tensor(
        "attn_part", [B, HEADS_PER_CORE, N, N], FP32, kind="ExternalOutput"
    )
    out_part = nc.dram_tensor("out_part", [B, N, C], FP16, kind="ExternalOutput")

    with tile.TileContext(nc) as tc:
        with (
            tc.tile_pool(name="persist", bufs=1) as persist,
            tc.tile_pool(name="stage", bufs=3) as stage,
            tc.tile_pool(name="dram", bufs=2, space="DRAM") as dram,
            tc.tile_pool(name="qkps", bufs=2, space="PSUM") as qkps,
            tc.tile_pool(name="pvps", bufs=1, space="PSUM") as pvps,
            tc.tile_pool(name="tpps", bufs=2, space="PSUM") as tpps,
        ):
            # ---- constants / weights ----
            identity = persist.tile([128, 128], FP16)
            make_identity(nc, identity[:, :])

            wproj_sb = persist.tile([128, C], FP16)
            nc.sync.dma_start(wproj_sb[:, :], wproj[:, :])

            # ---- persistent activations ----
            qT = [persist.tile([128, N], FP16, name=f"qT{b}") for b in range(B)]
            kT = [persist.tile([128, N], FP16, name=f"kT{b}") for b in range(B)]
            v_aug = [
                persist.tile([128, NJT, VAUG_W], FP16, name=f"vaug{b}")
                for b in range(B)
            ]
            exp_all = persist.tile([128, NJT, N], FP16)
            stacked_t = persist.tile([128, N], FP16)
            stacked = [stacked_t, stacked_t]
            qT2 = [persist.tile([128, N], FP16, name=f"qT2{b}") for b in range(B)]
            kT2 = [persist.tile([128, N], FP16, name=f"kT2{b}") for b in range(B)]
            recip_cols = [
                [
                    persist.tile([128, NIC], FP32, name=f"rcols{b}_{hh}")
                    for hh in range(HEADS_PER_CORE)
                ]
                for b in range(B)
            ]

            for b in range(B):
                for hh in range(HEADS_PER_CORE):
                    nc.vector.memset(v_aug[b][:, :, VAUG_H_OFF[hh] + 64], 1.0)

            # ================= projection phase =================
            # Accumulates through the shared tp PSUM pool (quarter tiles) so
            # proj(b1) overlaps batch-0 attention instead of serializing on
            # a dedicated pool.
            with tc.tile_pool(name="xtp", bufs=8) as xtp:
                w_sb = xtp.tile([128, NCT, 384], FP16, tag="w_sb", bufs=1)
                nc.sync.dma_start(
                    w_sb[:, :, :], wqkv.rearrange("(t p) w -> p t w", p=128)
                )
                for b in range(B):
                    xts = []
                    for ct in range(NCT):
                        xt = xtp.tile([128, N], FP16, tag="xt")
                        nc.sync.dma_start(xt[:, :], xT[b, ts(ct, 128), :])
                        xts.append(xt)

                    # qT, kT, vT accumulated in [128, 512] quarters
                    vT_sb = xtp.tile([128, N], FP16, tag="vT_sb", bufs=2)
                    for ti, dest in ((0, qT[b]), (1, kT[b]), (2, vT_sb)):
                        for qt in range(4):
                            ps = tpps.tile(
                                [128, 512], FP32, tag="tp", name=f"pj{b}_{ti}_{qt}"
                            )
                            for ct in range(NCT):
                                nc.tensor.matmul(
                                    ps[:, :],
                                    w_sb[:, ct, ts(ti, 128)],
                                    xts[ct][:, ts(qt, 512)],
                                    start=(ct == 0),
                                    stop=(ct == NCT - 1),
                                )
                            nc.scalar.copy(dest[:, ts(qt, 512)], ps[:, :])

                    # v_aug: PE-transpose vT 128x128 blocks into [j, d] layout
                    for jg in range(2):
                        tp = tpps.tile([128, 1024], FP16, tag="tp", name=f"vt{b}_{jg}")
                        for k in range(8):
                            jt = jg * 8 + k
                            nc.tensor.transpose(
                                tp[:, ts(k, 128)], vT_sb[:, ts(jt, 128)], identity[:, :]
                            )
                        for k in range(8):
                            jt = jg * 8 + k
                            for hh in range(HEADS_PER_CORE):
                                nc.vector.tensor_copy(
                                    v_aug[b][:, jt, ds(VAUG_H_OFF[hh], 64)],
                                    tp[:, ds(k * 128 + hh * 64, 64)],
                                )
                    # swapped-half copies so consecutive j-tiles use disjoint
                    # PE row groups (K=64 matmuls run concurrently)
                    for src_t, dst_t in ((qT[b], qT2[b]), (kT[b], kT2[b])):
                        nc.vector.tensor_copy(dst_t[0:64, :], src_t[64:128, :])
                        nc.vector.tensor_copy(dst_t[64:128, :], src_t[0:64, :])

            # ================= attention =================
            # PSUM: qk [128,1024]f32 x2 = 4 banks, pv [65,1024]f32 x1 = 2,
            # tp ([128,1024]fp16 | [128,512]f32) x2 = 2. QK j-tile pairs use
            # alternating PE row groups (via qT2/kT2 swapped copies) so the
            # K=64 matmuls pack 2x into the array.
            with tc.tile_pool(name="exp2p", bufs=1) as exp2p:
                exp_b = exp2p.tile([128, NJT, N], FP16)
                evac_cnt = 0
                for bh in range(B * HEADS_PER_CORE):
                    b, hh = divmod(bh, HEADS_PER_CORE)
                    ecur = exp_all if bh % 2 == 0 else exp_b

                    def qk_ops(jt):
                        par = jt % 2
                        row = ds(hh * 64 if par == 0 else (1 - hh) * 64, 64)
                        qsrc = qT[b] if par == 0 else qT2[b]
                        ksrc = kT[b] if par == 0 else kT2[b]
                        return ksrc[row, ts(jt, 128)], qsrc, row

                    # ---- pass A ----
                    pv0 = pvps.tile([65, 1024], FP32, tag="pv", name=f"pv0_{bh}")
                    for jp in range(NJT // 2):
                        jts = (2 * jp, 2 * jp + 1)
                        for ih in range(2):
                            qks = {}
                            for jt in jts:
                                qks[jt] = qkps.tile(
                                    [128, 1024], FP32, tag="qk", name=f"qk{bh}_{jt}_{ih}"
                                )
                            for q2 in range(2):
                                for jt in jts:
                                    klhs, qsrc, row = qk_ops(jt)
                                    nc.tensor.matmul(
                                        qks[jt][:, ts(q2, 512)],
                                        klhs,
                                        qsrc[row, ds(ih * 1024 + q2 * 512, 512)],
                                        start=True,
                                        stop=True,
                                    )
                            for jt in jts:
                                nc.scalar.activation(
                                    ecur[:, jt, ts(ih, 1024)],
                                    qks[jt][:, :],
                                    mybir.ActivationFunctionType.Exp,
                                    scale=float(SCALE),
                                )
                        for jt in jts:
                            for q2 in range(2):
                                nc.tensor.matmul(
                                    pv0[:, ts(q2, 512)],
                                    v_aug[b][:, jt, ds(VAUG_H_OFF[hh], 65)],
                                    ecur[:, jt, ts(q2, 512)],
                                    start=(jt == 0),
                                    stop=(jt == NJT - 1),
                                )

                    # ---- PV i-half 1 + denominators ----
                    sums_row = stage.tile([1, N], FP32, tag="sums_row", bufs=1)
                    nc.scalar.copy(sums_row[:, 0:1024], pv0[64:65, :])
                    nc.vector.tensor_copy(stacked[b][ds(hh * 64, 64), 0:1024], pv0[0:64, :])
                    pv1 = pvps.tile([65, 1024], FP32, tag="pv", name=f"pv1_{bh}")
                    for jt in range(NJT):
                        for q2 in range(2):
                            nc.tensor.matmul(
                                pv1[:, ts(q2, 512)],
                                v_aug[b][:, jt, ds(VAUG_H_OFF[hh], 65)],
                                ecur[:, jt, ds(1024 + q2 * 512, 512)],
                                start=(jt == 0),
                                stop=(jt == NJT - 1),
                            )
                    nc.scalar.copy(sums_row[:, 1024:2048], pv1[64:65, :])
                    nc.vector.tensor_copy(
                        stacked[b][ds(hh * 64, 64), 1024:2048], pv1[0:64, :]
                    )
                    rcols = recip_cols[b][hh]
                    bounce = dram.tile([1, N], FP32, tag="bounce")
                    nc.sync.dma_start(bounce[:, :], sums_row[:, :])
                    sums_cols = stage.tile([128, NIC], FP32, tag="sums_cols", bufs=2)
                    nc.sync.dma_start(
                        sums_cols[:, :],
                        bounce.rearrange("o (c p) -> (o p) c", p=128),
                    )
                    nc.vector.reciprocal(rcols[:, :], sums_cols[:, :])

                    # ---- output projection (tp pool, packed K=64 pairs) ----
                    if hh == 1:
                        for ic in range(NIC):
                            ps = []
                            for h2 in range(HEADS_PER_CORE):
                                p = pvps.tile(
                                    [128, 1024], FP32, tag="pv", name=f"op{b}_{ic}_{h2}"
                                )
                                for q2 in range(2):
                                    nc.tensor.matmul(
                                        p[:, ts(q2, 512)],
                                        stacked[b][ds(h2 * 64, 64), ts(ic, 128)],
                                        wproj_sb[ds(h2 * 64, 64), ts(q2, 512)],
                                        start=True,
                                        stop=True,
                                    )
                                ps.append(p)
                            optmp = stage.tile([128, C], FP16, tag="optmp", bufs=2)
                            if b == 1:
                                nc.scalar.mul(
                                    optmp[:, :],
                                    ps[0][:, :],
                                    recip_cols[b][0][:, ic : ic + 1],
                                )
                            else:
                                nc.vector.tensor_scalar_mul(
                                    optmp[:, :],
                                    ps[0][:, :],
                                    recip_cols[b][0][:, ic : ic + 1],
                                )
                            out_sb = stage.tile([128, C], FP16, tag="out_sb", bufs=2)
                            nc.vector.scalar_tensor_tensor(
                                out_sb[:, :],
                                ps[1][:, :],
                                recip_cols[b][1][:, ic : ic + 1],
                                optmp[:, :],
                                mybir.AluOpType.mult,
                                mybir.AluOpType.add,
                            )
                            nc.sync.dma_start(
                                out_part[b, ts(ic, 128), :], out_sb[:, :]
                            )

                    # ---- pass B ----
                    for ic in range(NIC):
                        attn_sb = stage.tile([128, N], FP16, tag="attn_sb", bufs=4)
                        for jh in range(2):
                            tp = tpps.tile(
                                [128, 1024], FP16, tag="tp", name=f"tp{bh}_{ic}_{jh}"
                            )
                            for k in range(8):
                                jt = jh * 8 + k
                                nc.tensor.transpose(
                                    tp[:, ts(k, 128)],
                                    ecur[:, jt, ts(ic, 128)],
                                    identity[:, :],
                                )
                            act_evac = (
                                evac_cnt % 2 == 1 if bh == 3 else evac_cnt % 4 == 3
                            )
                            if act_evac:
                                nc.scalar.mul(
                                    attn_sb[:, ts(jh, 1024)],
                                    tp[:, :],
                                    rcols[:, ic : ic + 1],
                                )
                            else:
                                nc.vector.tensor_scalar_mul(
                                    attn_sb[:, ts(jh, 1024)],
                                    tp[:, :],
                                    rcols[:, ic : ic + 1],
                                )
                            evac_cnt += 1
                        nc.gpsimd.dma_start(
                            attn_out[b, hh, ts(ic, 128), :], attn_sb[:, :]
                        )

    nc.finalize()
    return nc


def _prep_inputs(x, w_qkv, w_proj):
    xT = np.ascontiguousarray(x.transpose(0, 2, 1)).astype(np.float16)
    wq, wk, wv = w_qkv[0:C], w_qkv[C : 2 * C], w_qkv[2 * C : 3 * C]
    in_maps = []
    for c in range(N_CORES):
        sl = slice(c * 128, (c + 1) * 128)
        wqkv_c = np.concatenate(
            [wq[sl].T, wk[sl].T, wv[sl].T], axis=1
        ).astype(np.float16)
        wproj_c = np.ascontiguousarray(w_proj[:, sl].T).astype(np.float16)
        in_maps.append(
            {
                "xT": xT,
                "wqkv": np.ascontiguousarray(wqkv_c),
                "wproj": wproj_c,
            }
        )
    return in_maps


def _install_ntff_shim():
    """The agent image lacks antenv.axon_hooks; recreate the NTFF profile
    hook via ctypes against libaxon_pjrt.so (same ABI trn_boot uses)."""
    import contextlib
    import ctypes
    import sys
    import types

    try:
        from antenv.axon_hooks import get_axon_ntff_profile_hook  # noqa: F401

        return
    except ImportError:
        pass
    so_path = "/opt/axon/libaxon_pjrt.so"
    if not os.path.exists(so_path):
        return
    lib = ctypes.CDLL(so_path)
    if not hasattr(lib, "axon_start_nrt_profile"):
        return
    lib.axon_start_nrt_profile.argtypes = [
        ctypes.POINTER(ctypes.c_int64),
        ctypes.c_size_t,
    ]
    lib.axon_start_nrt_profile.restype = ctypes.c_int64
    lib.axon_stop_nrt_profile.argtypes = [ctypes.c_char_p]
    lib.axon_stop_nrt_profile.restype = ctypes.c_int64

    @contextlib.contextmanager
    def _hook(output_dir, device_ids):
        import jax

        jax.devices()
        if device_ids:
            ids = (ctypes.c_int64 * len(device_ids))(*device_ids)
            rc = lib.axon_start_nrt_profile(ids, len(device_ids))
        else:
            rc = lib.axon_start_nrt_profile(None, 0)
        if rc != 0:
            raise RuntimeError(f"axon_start_nrt_profile rc={rc}")
        try:
            yield
        finally:
            n = lib.axon_stop_nrt_profile(str(output_dir).encode())
            print(f"profile: {n} file(s) written to {output_dir}")

    mod = types.ModuleType("antenv.axon_hooks")
    mod.get_axon_ntff_profile_hook = lambda: _hook
    mod.set_axon_ntff_profile_hook = lambda h: None
    sys.modules["antenv.axon_hooks"] = mod


_CACHED = {}


def kernel(x, w_qkv, w_proj, b_proj):
    x = np.asarray(x)
    w_qkv = np.asarray(w_qkv)
    w_proj = np.asarray(w_proj)
    b_proj = np.asarray(b_proj)

    if "nc" not in _CACHED:
        _CACHED["nc"] = build_core_program()
    nc = _CACHED["nc"]

    in_maps = _prep_inputs(x, w_qkv, w_proj)
    trace = os.environ.get("KERNEL_TRACE", "") not in ("", "0")
    if trace:
        _install_ntff_shim()
        import concourse.bass_utils as _bu

        _bu.upload_artifacts = lambda d: d  # zero-egress container
    res = run_bass_kernel_spmd(
        nc, in_maps, core_ids=list(range(N_CORES)), trace=trace
    )
    if trace and res.exec_time_ns is not None:
        print(f"HW exec time: {res.exec_time_ns} ns")
        if res.instructions_and_trace:
            print(f"trace: {res.instructions_and_trace[1]}")

    attn = np.empty((B, H, N, N), dtype=np.float32)
    out = np.zeros((B, N, C), dtype=np.float32)
    for c in range(N_CORES):
        r = res.results[c]
        ap = r["attn_part"]
        for hh in range(HEADS_PER_CORE):
            attn[:, HEADS_PER_CORE * c + hh] = ap[:, hh]
        out += r["out_part"].astype(np.float32)
    out += b_proj.astype(np.float32)
    return out, attn
